# revision 1
# baseline (speedup 1.0000x reference)
"""Fused multi-head attention (B=4, N=2048, C=1024, H=16, D=64) on 8 NeuronCores.

Sharding: core i handles batch b = i // 2, head-group g = i % 2 (heads
8g..8g+7).  Each core runs an identical Bass/Tile program (SPMD) on its own
input shard:
  - qkv projection for its 1536 features (512 q + 512 k + 512 v), computed
    from host-pretransposed x[b].T and w.T so the contraction dim lands on
    SBUF partitions with contiguous DMA.
  - Q/K are produced directly in [feature, token] layout (what the S matmul
    wants); V in natural [token, feature] layout with an appended ones column
    per head (gives softmax denominators for free in the AV matmul).
  - Attention computes S.T = K.T' @ Q.T per head (scores transposed), exp on
    ScalarE (no max subtraction: |S| <= ~10, well inside fp32/bf16 range),
    AV accumulation in PSUM, then a PE transpose + per-row 1/sum scaling
    produces the output in natural layout.
All matmuls run in bf16 with fp32 PSUM accumulation.
"""

from contextlib import ExitStack

import ml_dtypes
import numpy as np

import concourse.bass as bass
import concourse.mybir as mybir
import concourse.tile as tile
from concourse import bacc
from concourse.bass_utils import run_bass_kernel_spmd
from concourse.masks import make_identity

dt = mybir.dt
AF = mybir.ActivationFunctionType
BF16 = dt.bfloat16
F32 = dt.float32

B, N_TOK, C_IN = 4, 2048, 1024
NH = 8            # heads per core
NPAIR = NH // 2   # heads processed as row-packed pairs in the S matmul
D = 64
WF = 1536         # projected features per core (512 q + 512 k + 512 v)
KC = C_IN // 128  # contraction k-tiles
MT = N_TOK // 128 # token tiles (m / output row chunks)
TB = N_TOK // 512 # 512-wide token blocks for the projection
VROW = 65         # V columns per head incl. ones column


def build_nc(iters: int = 1):
    nc = bacc.Bacc(trn_type="TRN2")
    xT = nc.dram_tensor("xT", [C_IN, N_TOK], BF16, kind="ExternalInput").ap()
    wT = nc.dram_tensor("wT", [C_IN, WF], BF16, kind="ExternalInput").ap()
    qkb = nc.dram_tensor("qkb", [1024], F32, kind="ExternalInput").ap()
    vb = nc.dram_tensor("vb", [512], F32, kind="ExternalInput").ap()
    out = nc.dram_tensor("out", [N_TOK, NH * D], F32, kind="ExternalOutput").ap()

    with tile.TileContext(nc) as tc, ExitStack() as ctx:
        consts = ctx.enter_context(tc.tile_pool(name="consts", bufs=1))
        p_xt = ctx.enter_context(tc.tile_pool(name="p_xt", bufs=KC))
        p_wt = ctx.enter_context(tc.tile_pool(name="p_wt", bufs=KC))
        p_qkt = ctx.enter_context(tc.tile_pool(name="p_qkt", bufs=2 * NPAIR))
        p_vp = ctx.enter_context(tc.tile_pool(name="p_vp", bufs=MT))
        p_pt = ctx.enter_context(tc.tile_pool(name="p_pt", bufs=1))
        p_osb = ctx.enter_context(tc.tile_pool(name="p_osb", bufs=2))
        p_eps = ctx.enter_context(tc.tile_pool(name="p_eps", bufs=4))

        identity = consts.tile([128, 128], BF16, name="identity")
        make_identity(nc, identity)
        qkb_sb = consts.tile([128, 8], F32, name="qkb_sb")
        nc.sync.dma_start(out=qkb_sb, in_=qkb.rearrange("(t p) -> p t", p=128))
        vb_bc = consts.tile([128, 512], F32, name="vb_bc")
        nc.sync.dma_start(
            out=vb_bc,
            in_=bass.AP(tensor=vb.tensor, offset=vb.offset, ap=[[0, 128], vb.ap[0]]),
        )

        def body():
            xt, wt = [], []
            for kc in range(KC):
                tx = p_xt.tile([128, N_TOK], BF16, name=f"xt{kc}", tag="xt")
                nc.sync.dma_start(out=tx, in_=xT[kc * 128 : (kc + 1) * 128, :])
                xt.append(tx)
                tw = p_wt.tile([128, WF], BF16, name=f"wt{kc}", tag="wt")
                nc.sync.dma_start(out=tw, in_=wT[kc * 128 : (kc + 1) * 128, :])
                wt.append(tw)

            qkt = [
                p_qkt.tile([128, N_TOK], BF16, name=f"qkt{ft}", tag="qkt")
                for ft in range(2 * NPAIR)
            ]
            vp = [
                p_vp.tile([128, NH * VROW], BF16, name=f"vp{tt}", tag="vp")
                for tt in range(MT)
            ]

            def proj_qk(ft, pool, tag):
                # qkt[ft] = (x @ w[ft-block].T + b).T  -> [feature, token]
                for tb in range(TB):
                    ps = pool.tile([128, 512], F32, name=f"pj{ft}_{tb}", tag=tag)
                    for kc in range(KC):
                        nc.tensor.matmul(
                            ps,
                            lhsT=wt[kc][:, ft * 128 : (ft + 1) * 128],
                            rhs=xt[kc][:, tb * 512 : (tb + 1) * 512],
                            start=(kc == 0),
                            stop=(kc == KC - 1),
                        )
                    nc.vector.tensor_scalar_add(
                        out=qkt[ft][:, tb * 512 : (tb + 1) * 512],
                        in0=ps,
                        scalar1=qkb_sb[:, ft : ft + 1],
                    )

            def proj_v(tt, pool, tag):
                # vp[tt][:, h*65:h*65+64] = x-tile @ w_v[h].T + b_v[h]; col h*65+64 = 1
                ps = pool.tile([128, 512], F32, name=f"pv{tt}", tag=tag)
                for kc in range(KC):
                    nc.tensor.matmul(
                        ps,
                        lhsT=xt[kc][:, tt * 128 : (tt + 1) * 128],
                        rhs=wt[kc][:, 1024:1536],
                        start=(kc == 0),
                        stop=(kc == KC - 1),
                    )
                t = vp[tt]
                nc.gpsimd.memset(t, 1.0)
                for h in range(NH):
                    nc.vector.tensor_add(
                        out=t[:, h * VROW : h * VROW + 64],
                        in0=ps[:, h * 64 : (h + 1) * 64],
                        in1=vb_bc[:, h * 64 : (h + 1) * 64],
                    )

            # ---- phase A: first pair's Q/K projection, then V projection ----
            with tc.tile_pool(name="pp_proj", bufs=4, space="PSUM") as pp_proj:
                proj_qk(0, pp_proj, "pj")
                proj_qk(NPAIR, pp_proj, "pj")
                for tt in range(MT):
                    proj_v(tt, pp_proj, "pj")

            # ---- phase B/C: attention pairs with trickled proj + epilogue ----
            # PSUM budget (8 banks): sA 2 + sB 2 + avA 2 + b0 1 + b1 1.
            # Head B accumulates inline into two single-bank tiles (b0/b1) so
            # no slot ring couples head A's and head B's accumulators; the
            # trickled projection rides the b0/b1 rings between pairs and the
            # epilogue transposes ride the avA ring.
            with tc.tile_pool(name="pp_s", bufs=1, space="PSUM") as pp_s, \
                 tc.tile_pool(name="pp_av", bufs=1, space="PSUM") as pp_av, \
                 tc.tile_pool(name="pp_b", bufs=1, space="PSUM") as pp_b:

                def epilogue_half(o_t, h, half):
                    # transpose each 128-token chunk to [token, d'], then rows /= sums
                    for chk in range(8 * half, 8 * (half + 1)):
                        tr = pp_av.tile([128, VROW], BF16, name="tr", tag="av")
                        nc.tensor.transpose(
                            tr,
                            in_=o_t[:, chk * 128 : (chk + 1) * 128],
                            identity=identity[0:VROW, 0:VROW],
                        )
                        rc = p_eps.tile([128, 1], F32, name="rc", tag="rc", bufs=4)
                        nc.vector.reciprocal(out=rc, in_=tr[:, 64:65])
                        ob = p_eps.tile([128, 64], F32, name="ob", tag="ob", bufs=4)
                        nc.vector.tensor_scalar_mul(out=ob, in0=tr[:, 0:64], scalar1=rc)
                        nc.sync.dma_start(
                            out=out[chk * 128 : (chk + 1) * 128, h * 64 : (h + 1) * 64],
                            in_=ob,
                        )

                epilogues = []
                for p in range(NPAIR):
                    o_sb = [
                        p_osb.tile([VROW, N_TOK], BF16, name=f"osb{p}_{hh}", tag=f"o{hh}")
                        for hh in range(2)
                    ]
                    ha, hb = 2 * p, 2 * p + 1
                    for half in range(2):
                        n0 = half * 1024
                        av_a = pp_av.tile([VROW, 1024], F32, name="av_a", tag="av")
                        av_b = [
                            pp_b.tile([VROW, 512], F32, name=f"av_b{nb}", tag=f"b{nb}")
                            for nb in range(2)
                        ]
                        for m in range(MT):
                            s_a = pp_s.tile([128, 1024], F32, name="s_a", tag="sA")
                            s_b = pp_s.tile([128, 1024], F32, name="s_b", tag="sB")
                            for nb in range(2):
                                nsl = slice(n0 + nb * 512, n0 + (nb + 1) * 512)
                                nc.tensor.matmul(
                                    s_a[:, nb * 512 : (nb + 1) * 512],
                                    lhsT=qkt[NPAIR + p][0:64, m * 128 : (m + 1) * 128],
                                    rhs=qkt[p][0:64, nsl],
                                    start=True,
                                    stop=True,
                                )
                                nc.tensor.matmul(
                                    s_b[:, nb * 512 : (nb + 1) * 512],
                                    lhsT=qkt[NPAIR + p][64:128, m * 128 : (m + 1) * 128],
                                    rhs=qkt[p][64:128, nsl],
                                    start=True,
                                    stop=True,
                                )
                            pt_a = p_pt.tile([128, 1024], BF16, name="pt_a", tag="ptA", bufs=8)
                            nc.scalar.activation(out=pt_a, in_=s_a, func=AF.Exp, scale=0.125)
                            pt_b = p_pt.tile([128, 1024], BF16, name="pt_b", tag="ptB", bufs=8)
                            nc.scalar.activation(out=pt_b, in_=s_b, func=AF.Exp, scale=0.125)
                            for nb in range(2):
                                nc.tensor.matmul(
                                    av_a[:, nb * 512 : (nb + 1) * 512],
                                    lhsT=vp[m][:, ha * VROW : ha * VROW + VROW],
                                    rhs=pt_a[:, nb * 512 : (nb + 1) * 512],
                                    start=(m == 0),
                                    stop=(m == MT - 1),
                                )
                                nc.tensor.matmul(
                                    av_b[nb],
                                    lhsT=vp[m][:, hb * VROW : hb * VROW + VROW],
                                    rhs=pt_b[:, nb * 512 : (nb + 1) * 512],
                                    start=(m == 0),
                                    stop=(m == MT - 1),
                                )
                        nc.vector.tensor_copy(out=o_sb[0][:, n0 : n0 + 1024], in_=av_a)
                        for nb in range(2):
                            nc.vector.tensor_copy(
                                out=o_sb[1][:, n0 + nb * 512 : n0 + (nb + 1) * 512],
                                in_=av_b[nb],
                            )
                        for hh in range(2):
                            epilogue_half(o_sb[hh], 2 * p + hh, half)

                    # trickle next pair's Q/K projection into PE gaps (psum via
                    # the b0/b1 rings, which are idle between accumulations)
                    if p + 1 < NPAIR:
                        proj_qk(p + 1, pp_b, "b0")
                        proj_qk(NPAIR + p + 1, pp_b, "b1")

        for _ in range(iters):
            body()

    nc.finalize()
    return nc


_NC_CACHE = {}


def _get_nc(iters: int = 1):
    if iters not in _NC_CACHE:
        _NC_CACHE[iters] = build_nc(iters)
    return _NC_CACHE[iters]


def make_in_maps(x, qkv_w, qkv_b):
    bf = ml_dtypes.bfloat16
    in_maps = []
    for core in range(8):
        b, g = core // 2, core % 2
        xTc = np.ascontiguousarray(x[b].T).astype(bf)
        wq = qkv_w[g * 512 : (g + 1) * 512]
        wk = qkv_w[1024 + g * 512 : 1024 + (g + 1) * 512]
        wv = qkv_w[2048 + g * 512 : 2048 + (g + 1) * 512]
        wTc = np.ascontiguousarray(np.concatenate([wq, wk, wv], axis=0).T).astype(bf)
        qkbc = np.ascontiguousarray(
            np.concatenate(
                [qkv_b[g * 512 : (g + 1) * 512], qkv_b[1024 + g * 512 : 1024 + (g + 1) * 512]]
            )
        ).astype(np.float32)
        vbc = np.ascontiguousarray(qkv_b[2048 + g * 512 : 2048 + (g + 1) * 512]).astype(
            np.float32
        )
        in_maps.append({"xT": xTc, "wT": wTc, "qkb": qkbc, "vb": vbc})
    return in_maps


_RUNNER_CACHE = {}


def _get_runner(iters: int = 1, n_cores: int = 8):
    """Build the shard_map-wrapped bass_exec executable once and reuse it, so
    repeated kernel() calls don't re-ship the NEFF through the axon tunnel."""
    if iters in _RUNNER_CACHE:
        return _RUNNER_CACHE[iters]
    import jax
    from jax.sharding import Mesh, PartitionSpec
    from jax.experimental.shard_map import shard_map
    from concourse.bass2jax import (
        _bass_exec_p,
        install_neuronx_cc_hook,
        partition_id_tensor,
    )

    nc = _get_nc(iters)
    install_neuronx_cc_hook()
    partition_name = nc.partition_id_tensor.name if nc.partition_id_tensor else None
    in_names, out_names, out_avals, zero_outs = [], [], [], []
    for alloc in nc.m.functions[0].allocations:
        if not isinstance(alloc, mybir.MemoryLocationSet):
            continue
        name = alloc.memorylocations[0].name
        if alloc.kind == "ExternalInput":
            if name != partition_name:
                in_names.append(name)
        elif alloc.kind == "ExternalOutput":
            shape = tuple(alloc.tensor_shape)
            npdt = dt.np(alloc.dtype)
            out_names.append(name)
            out_avals.append(jax.core.ShapedArray(shape, npdt))
            zero_outs.append(np.zeros(shape, npdt))
    n_params = len(in_names)
    all_in_names = list(in_names) + list(out_names)
    if partition_name is not None:
        all_in_names.append(partition_name)

    def _body(*args):
        operands = list(args)
        if partition_name is not None:
            operands.append(partition_id_tensor())
        return tuple(
            _bass_exec_p.bind(
                *operands,
                out_avals=tuple(out_avals),
                in_names=tuple(all_in_names),
                out_names=tuple(out_names),
                lowering_input_output_aliases=(),
                sim_require_finite=True,
                sim_require_nnan=True,
                nc=nc,
            )
        )

    devices = jax.devices()[:n_cores]
    mesh = Mesh(np.asarray(devices), ("core",))
    in_specs = (PartitionSpec("core"),) * (n_params + len(out_names))
    out_specs = (PartitionSpec("core"),) * len(out_names)
    fn = jax.jit(
        shard_map(_body, mesh=mesh, in_specs=in_specs, out_specs=out_specs, check_rep=False)
    )
    zero_concat = [
        np.zeros((n_cores * z.shape[0], *z.shape[1:]), z.dtype) for z in zero_outs
    ]
    _RUNNER_CACHE[iters] = (fn, in_names, zero_concat, mesh)
    return _RUNNER_CACHE[iters]


def kernel(x, qkv_w, qkv_b):
    import jax

    x = np.asarray(x, dtype=np.float32)
    qkv_w = np.asarray(qkv_w, dtype=np.float32)
    qkv_b = np.asarray(qkv_b, dtype=np.float32)
    in_maps = make_in_maps(x, qkv_w, qkv_b)
    fn, in_names, zero_concat, _ = _get_runner(1)
    concat_in = [
        np.concatenate([in_maps[c][name] for c in range(8)], axis=0) for name in in_names
    ]
    outs = fn(*concat_in, *zero_concat)
    out_global = np.asarray(jax.block_until_ready(outs)[0])
    full = np.empty((B, N_TOK, C_IN), dtype=np.float32)
    for core in range(8):
        b, g = core // 2, core % 2
        full[b, :, g * 512 : (g + 1) * 512] = out_global[core * N_TOK : (core + 1) * N_TOK]
    return full



# revision 13
# speedup vs baseline: 1.9337x; 1.9337x over previous
"""Fused multi-head attention (B=4, N=2048, C=1024, H=16, D=64) on 8 NeuronCores.

Sharding: core i handles batch b = i // 2, head-group g = i % 2 (heads
8g..8g+7).  Each core runs an identical Bass/Tile program (SPMD).

v2 schedule: the ScalarE exp stream (256 x [128,1024] activations) is the
binding engine together with PE; the program is organized as one flat loop of
256 attention steps (pair p, 512-token window w, key-chunk m) so that:
  - exp starts ~15us in: only pair-0's Q/K projection gates it (weights are
    host-split so those columns DMA first);
  - all remaining projection work (Q/K of pairs 1-3, all of V) trickles into
    per-step PE slack through a dedicated 1-bank PSUM ring;
  - AV matmuls trail the exp stream by 12 steps (pt ring keeps 16 tiles), so
    V-projection in the first 16 steps never stalls ScalarE;
  - S matmuls for the two heads of a pair use PE row-tiles (0,0)/(64,0)
    (K=64 each) so they can run concurrently on hardware.
Scores are computed transposed (S.T = K.T' @ Q.T per head); exp on ScalarE
(no max subtraction: |S|*0.125 <= ~6); V carries an appended ones column so
softmax denominators fall out of the AV matmul; a PE transpose + per-row
1/den scaling produces the output in natural layout.  All matmuls run in
bf16 with fp32 PSUM accumulation.
"""

from contextlib import ExitStack

import ml_dtypes
import numpy as np

import concourse.bass as bass
import concourse.mybir as mybir
import concourse.tile as tile
from concourse import bacc
from concourse.masks import make_identity

dt = mybir.dt
AF = mybir.ActivationFunctionType
BF16 = dt.bfloat16
F32 = dt.float32

B, N_TOK, C_IN = 4, 2048, 1024
NH = 8            # heads per core
NPAIR = NH // 2   # head pairs
D = 64
KC = C_IN // 128  # contraction k-tiles
MT = N_TOK // 128 # key-token chunks per window loop
NW = N_TOK // 512 # 512-token output windows per pair
VROW = 65         # V columns per head incl. ones column
TRAIL = 28        # AV trails exp by this many flat steps
NSTEP = NPAIR * NW * MT  # 256 flat attention steps


def build_nc(iters: int = 1):
    nc = bacc.Bacc(trn_type="TRN2")
    xT = nc.dram_tensor("xT", [C_IN, N_TOK], BF16, kind="ExternalInput").ap()
    wT1 = nc.dram_tensor("wT1", [C_IN, 256], BF16, kind="ExternalInput").ap()
    wT2 = nc.dram_tensor("wT2", [C_IN, 768], BF16, kind="ExternalInput").ap()
    wTv = nc.dram_tensor("wTv", [C_IN, 512], BF16, kind="ExternalInput").ap()
    qkb = nc.dram_tensor("qkb", [1024], F32, kind="ExternalInput").ap()
    vb = nc.dram_tensor("vb", [512], F32, kind="ExternalInput").ap()
    out = nc.dram_tensor("out", [N_TOK, NH * D], F32, kind="ExternalOutput").ap()

    with tile.TileContext(nc) as tc, ExitStack() as ctx:
        consts = ctx.enter_context(tc.tile_pool(name="consts", bufs=1))
        p_xt = ctx.enter_context(tc.tile_pool(name="p_xt", bufs=1))
        p_w1 = ctx.enter_context(tc.tile_pool(name="p_w1", bufs=1))
        p_w2 = ctx.enter_context(tc.tile_pool(name="p_w2", bufs=1))
        p_wv = ctx.enter_context(tc.tile_pool(name="p_wv", bufs=1))
        p_qk = ctx.enter_context(tc.tile_pool(name="p_qk", bufs=2 * NPAIR))
        p_vp = ctx.enter_context(tc.tile_pool(name="p_vp", bufs=MT))
        p_pt = ctx.enter_context(tc.tile_pool(name="p_pt", bufs=32))
        p_osb = ctx.enter_context(tc.tile_pool(name="p_osb", bufs=4))
        p_eps = ctx.enter_context(tc.tile_pool(name="p_eps", bufs=4))

        identity = consts.tile([128, 128], BF16, name="identity")
        make_identity(nc, identity)
        qkb_sb = consts.tile([128, 8], F32, name="qkb_sb")
        nc.sync.dma_start(out=qkb_sb, in_=qkb.rearrange("(t p) -> p t", p=128))
        vb_bc = consts.tile([128, 512], F32, name="vb_bc")
        nc.sync.dma_start(
            out=vb_bc,
            in_=bass.AP(tensor=vb.tensor, offset=vb.offset, ap=[[0, 128], vb.ap[0]]),
        )
        # preload the exp table set while input DMAs run
        warm = consts.tile([128, 1], F32, name="warm")
        nc.scalar.activation(out=warm, in_=qkb_sb[:, 0:1], func=AF.Exp)

        def body():
            # ---- input DMAs: one multi-dim-AP DMA per tensor (queue/SEQ
            # cost per dma_start is ~650ns; merging is a big head saving),
            # ordered by first use: w1+xtA gate the first S, wv gates the
            # trickled V units (steps 6+), xtB gates kU(0,2/3) (steps 2/4),
            # w2 gates pair-1 units (step 22+). ----
            w1t = p_w1.tile([128, KC * 256], BF16, name="w1", tag="w1")
            nc.sync.dma_start(
                out=w1t.rearrange("p (kc c) -> p kc c", kc=KC),
                in_=wT1.rearrange("(kc p) c -> p kc c", p=128),
            )
            xtt = p_xt.tile([128, KC * N_TOK], BF16, name="xt", tag="xt")
            xt3_sb = xtt.rearrange("p (kc c) -> p kc c", kc=KC)
            xt3_hbm = xT.rearrange("(kc p) c -> p kc c", p=128)
            nc.sync.dma_start(out=xt3_sb[:, :, 0:1024], in_=xt3_hbm[:, :, 0:1024])
            wvt = p_wv.tile([128, KC * 512], BF16, name="wv", tag="wv")
            nc.sync.dma_start(
                out=wvt.rearrange("p (kc c) -> p kc c", kc=KC),
                in_=wTv.rearrange("(kc p) c -> p kc c", p=128),
            )
            nc.sync.dma_start(out=xt3_sb[:, :, 1024:2048], in_=xt3_hbm[:, :, 1024:2048])
            w2t = p_w2.tile([128, KC * 768], BF16, name="w2", tag="w2")
            nc.sync.dma_start(
                out=w2t.rearrange("p (kc c) -> p kc c", kc=KC),
                in_=wT2.rearrange("(kc p) c -> p kc c", p=128),
            )

            def xts(kc, sl):
                return xtt[:, kc * N_TOK + sl.start : kc * N_TOK + sl.stop]

            wv = [wvt[:, kc * 512 : (kc + 1) * 512] for kc in range(KC)]

            # persistent SBUF tiles
            qk = [
                p_qk.tile([128, N_TOK], BF16, name=f"qk{u}", tag="qk")
                for u in range(2 * NPAIR)
            ]
            vp = [
                p_vp.tile([128, NH * VROW], BF16, name=f"vp{tt}", tag="vp")
                for tt in range(MT)
            ]

            def w_slice(u, kc):
                # unit u: 0=q0,1=k0 from wT1; 2..7 = q1,k1,q2,k2,q3,k3 from wT2
                if u < 2:
                    return w1t[:, kc * 256 + u * 128 : kc * 256 + (u + 1) * 128]
                base = kc * 768 + (u - 2) * 128
                return w2t[:, base : base + 128]

            def qk_unit(pp_proj, u, tb):
                pj = pp_proj.tile([128, 512], F32, name=f"pj{u}_{tb}", tag="pj")
                for kc in range(KC):
                    nc.tensor.matmul(
                        pj,
                        lhsT=w_slice(u, kc),
                        rhs=xts(kc, slice(tb * 512, (tb + 1) * 512)),
                        start=(kc == 0),
                        stop=(kc == KC - 1),
                    )
                nc.vector.tensor_scalar_add(
                    out=qk[u][:, tb * 512 : (tb + 1) * 512],
                    in0=pj,
                    scalar1=qkb_sb[:, u : u + 1],
                )

            def v_unit(pp_proj, tt):
                pj = pp_proj.tile([128, 512], F32, name=f"pv{tt}", tag="pj")
                for kc in range(KC):
                    nc.tensor.matmul(
                        pj,
                        lhsT=xts(kc, slice(tt * 128, (tt + 1) * 128)),
                        rhs=wv[kc],
                        start=(kc == 0),
                        stop=(kc == KC - 1),
                    )
                t = vp[tt]
                t3 = t.rearrange("p (h d) -> p h d", h=NH)
                nc.gpsimd.memset(t3[:, :, 64:65], 1.0)
                nc.vector.tensor_add(
                    out=t3[:, :, 0:64],
                    in0=pj.rearrange("p (h d) -> p h d", h=NH),
                    in1=vb_bc.rearrange("p (h d) -> p h d", h=NH),
                )

            # trickle schedule: flat step -> list of (kind, args)
            emits = {}

            def emit_at(i, item):
                emits.setdefault(i, []).append(item)

            emit_at(2, ("qk", 1, 2))
            emit_at(4, ("qk", 1, 3))
            for t in range(MT):
                emit_at(6 + 2 * t, ("v", t))  # even steps 6..36; deadline t+TRAIL
            emit_at(9, ("qk", 0, 1))
            emit_at(25, ("qk", 0, 2))
            emit_at(41, ("qk", 0, 3))
            for p in range(1, NPAIR):
                for w in range(1, NW):
                    emit_at(64 * p + 16 * w - 8, ("qk", 2 * p, w))
                emit_at(64 * p - 41, ("qk", 2 * p, 0))
                for tb in range(NW):
                    emit_at(64 * p - 33 + 8 * tb, ("qk", 2 * p + 1, tb))

            with tc.tile_pool(name="pp_s", bufs=2, space="PSUM") as pp_s, \
                 tc.tile_pool(name="pp_av", bufs=1, space="PSUM") as pp_av, \
                 tc.tile_pool(name="pp_proj", bufs=1, space="PSUM") as pp_proj, \
                 tc.tile_pool(name="pp_tr", bufs=1, space="PSUM") as pp_tr:

                def epilogue(k, av_t):
                    p, w = divmod(k, NW)
                    for hh in range(2):
                        h = 2 * p + hh
                        osb = p_osb.tile([VROW, 512], BF16, name=f"osb{k}_{hh}", tag="osb")
                        nc.vector.tensor_copy(out=osb, in_=av_t[hh])
                        ob = p_eps.tile([128, 256], F32, name="ob", tag="ob")
                        for c in range(4):
                            tr = pp_tr.tile([128, VROW], BF16, name="tr", tag="tr")
                            nc.tensor.transpose(
                                tr,
                                in_=osb[:, c * 128 : (c + 1) * 128],
                                identity=identity[0:VROW, 0:VROW],
                            )
                            rc = p_eps.tile([128, 1], F32, name="rc", tag="rc")
                            nc.vector.reciprocal(out=rc, in_=tr[:, 64:65])
                            nc.vector.tensor_scalar_mul(
                                out=ob[:, c * 64 : (c + 1) * 64],
                                in0=tr[:, 0:64],
                                scalar1=rc,
                            )
                        # one DMA for the whole [512-token, 64-dim] block,
                        # issued from the Activation HWDGE queue (SP's is busy)
                        dst = out[w * 512 : (w + 1) * 512, h * 64 : (h + 1) * 64]
                        nc.scalar.dma_start(
                            out=dst.rearrange("(c p) d -> p c d", p=128),
                            in_=ob.rearrange("p (c d) -> p c d", c=4),
                        )

                # head units: pair-0 Q window 0 + K for tokens 0:1024
                qk_unit(pp_proj, 0, 0)
                qk_unit(pp_proj, 1, 0)
                qk_unit(pp_proj, 1, 1)

                pt_ring = [None] * NSTEP
                av_by_win = {}

                def av_step(j):
                    p, rem = divmod(j, NW * MT)
                    w, m = divmod(rem, MT)
                    k = p * NW + w
                    if m == 0:
                        av_by_win[k] = [
                            pp_av.tile([VROW, 512], F32, name=f"av{k}_{hh}",
                                       tag=f"av{hh}")
                            for hh in range(2)
                        ]
                    av_t = av_by_win[k]
                    pt = pt_ring[j]
                    for hh in range(2):
                        h = 2 * p + hh
                        nc.tensor.matmul(
                            av_t[hh],
                            lhsT=vp[m][:, h * VROW : (h + 1) * VROW],
                            rhs=pt[:, hh * 512 : (hh + 1) * 512],
                            start=(m == 0),
                            stop=(m == MT - 1),
                        )
                    pt_ring[j] = None
                    if m == MT - 1:
                        epilogue(k, av_t)

                for i in range(NSTEP):
                    p, rem = divmod(i, NW * MT)
                    w, m = divmod(rem, MT)
                    # S + exp for step i
                    s = pp_s.tile([128, 1024], F32, name="s", tag="s")
                    for hh in range(2):
                        rows = slice(hh * 64, (hh + 1) * 64)
                        nc.tensor.matmul(
                            s[:, hh * 512 : (hh + 1) * 512],
                            lhsT=qk[2 * p + 1][rows, m * 128 : (m + 1) * 128],
                            rhs=qk[2 * p][rows, w * 512 : (w + 1) * 512],
                            start=True,
                            stop=True,
                            tile_position=(hh * 64, 0),
                        )
                    pt = p_pt.tile([128, 1024], BF16, name=f"pt{i % 16}", tag="pt")
                    nc.scalar.activation(out=pt, in_=s, func=AF.Exp, scale=0.125)
                    pt_ring[i] = pt
                    # trickled projection work
                    for item in emits.get(i, ()):
                        if item[0] == "v":
                            v_unit(pp_proj, item[1])
                        else:
                            qk_unit(pp_proj, item[1], item[2])
                    # trailing AV
                    if i >= TRAIL:
                        av_step(i - TRAIL)
                for j in range(NSTEP - TRAIL, NSTEP):
                    av_step(j)

        for _ in range(iters):
            body()

    nc.finalize()
    return nc


_NC_CACHE = {}


def _get_nc(iters: int = 1):
    if iters not in _NC_CACHE:
        _NC_CACHE[iters] = build_nc(iters)
    return _NC_CACHE[iters]


def make_in_maps(x, qkv_w, qkv_b):
    bf = ml_dtypes.bfloat16
    in_maps = []
    for core in range(8):
        b, g = core // 2, core % 2
        xTc = np.ascontiguousarray(x[b].T).astype(bf)
        wq = qkv_w[g * 512 : (g + 1) * 512]
        wk = qkv_w[1024 + g * 512 : 1024 + (g + 1) * 512]
        wv = qkv_w[2048 + g * 512 : 2048 + (g + 1) * 512]
        bq = qkv_b[g * 512 : (g + 1) * 512]
        bk = qkv_b[1024 + g * 512 : 1024 + (g + 1) * 512]
        bv = qkv_b[2048 + g * 512 : 2048 + (g + 1) * 512]
        w1 = np.concatenate([wq[0:128], wk[0:128]], axis=0)
        w2 = np.concatenate(
            [arr for f in range(1, 4) for arr in (wq[f * 128 : (f + 1) * 128],
                                                  wk[f * 128 : (f + 1) * 128])],
            axis=0,
        )
        qkbc = np.concatenate(
            [arr for f in range(4) for arr in (bq[f * 128 : (f + 1) * 128],
                                               bk[f * 128 : (f + 1) * 128])]
        )
        in_maps.append(
            {
                "xT": xTc,
                "wT1": np.ascontiguousarray(w1.T).astype(bf),
                "wT2": np.ascontiguousarray(w2.T).astype(bf),
                "wTv": np.ascontiguousarray(wv.T).astype(bf),
                "qkb": np.ascontiguousarray(qkbc).astype(np.float32),
                "vb": np.ascontiguousarray(bv).astype(np.float32),
            }
        )
    return in_maps


_RUNNER_CACHE = {}


def _get_runner(iters: int = 1, n_cores: int = 8):
    """Build the shard_map-wrapped bass_exec executable once and reuse it, so
    repeated kernel() calls don't re-ship the NEFF through the axon tunnel."""
    if iters in _RUNNER_CACHE:
        return _RUNNER_CACHE[iters]
    import jax
    from jax.sharding import Mesh, PartitionSpec
    from jax.experimental.shard_map import shard_map
    from concourse.bass2jax import (
        _bass_exec_p,
        install_neuronx_cc_hook,
        partition_id_tensor,
    )

    nc = _get_nc(iters)
    install_neuronx_cc_hook()
    partition_name = nc.partition_id_tensor.name if nc.partition_id_tensor else None
    in_names, out_names, out_avals, zero_outs = [], [], [], []
    for alloc in nc.m.functions[0].allocations:
        if not isinstance(alloc, mybir.MemoryLocationSet):
            continue
        name = alloc.memorylocations[0].name
        if alloc.kind == "ExternalInput":
            if name != partition_name:
                in_names.append(name)
        elif alloc.kind == "ExternalOutput":
            shape = tuple(alloc.tensor_shape)
            npdt = dt.np(alloc.dtype)
            out_names.append(name)
            out_avals.append(jax.core.ShapedArray(shape, npdt))
            zero_outs.append(np.zeros(shape, npdt))
    n_params = len(in_names)
    all_in_names = list(in_names) + list(out_names)
    if partition_name is not None:
        all_in_names.append(partition_name)

    def _body(*args):
        operands = list(args)
        if partition_name is not None:
            operands.append(partition_id_tensor())
        return tuple(
            _bass_exec_p.bind(
                *operands,
                out_avals=tuple(out_avals),
                in_names=tuple(all_in_names),
                out_names=tuple(out_names),
                lowering_input_output_aliases=(),
                sim_require_finite=True,
                sim_require_nnan=True,
                nc=nc,
            )
        )

    devices = jax.devices()[:n_cores]
    mesh = Mesh(np.asarray(devices), ("core",))
    in_specs = (PartitionSpec("core"),) * (n_params + len(out_names))
    out_specs = (PartitionSpec("core"),) * len(out_names)
    fn = jax.jit(
        shard_map(_body, mesh=mesh, in_specs=in_specs, out_specs=out_specs, check_rep=False)
    )
    zero_concat = [
        np.zeros((n_cores * z.shape[0], *z.shape[1:]), z.dtype) for z in zero_outs
    ]
    _RUNNER_CACHE[iters] = (fn, in_names, zero_concat, mesh)
    return _RUNNER_CACHE[iters]


def kernel(x, qkv_w, qkv_b):
    import jax

    x = np.asarray(x, dtype=np.float32)
    qkv_w = np.asarray(qkv_w, dtype=np.float32)
    qkv_b = np.asarray(qkv_b, dtype=np.float32)
    in_maps = make_in_maps(x, qkv_w, qkv_b)
    fn, in_names, zero_concat, _ = _get_runner(1)
    concat_in = [
        np.concatenate([in_maps[c][name] for c in range(8)], axis=0) for name in in_names
    ]
    outs = fn(*concat_in, *zero_concat)
    out_global = np.asarray(jax.block_until_ready(outs)[0])
    full = np.empty((B, N_TOK, C_IN), dtype=np.float32)
    for core in range(8):
        b, g = core // 2, core % 2
        full[b, :, g * 512 : (g + 1) * 512] = out_global[core * N_TOK : (core + 1) * N_TOK]
    return full


# revision 18
# speedup vs baseline: 1.9484x; 1.0076x over previous
"""Fused multi-head attention (B=4, N=2048, C=1024, H=16, D=64) on 8 NeuronCores.

Sharding: core i handles batch b = i // 2, head-group g = i % 2 (heads
8g..8g+7).  Each core runs an identical Bass/Tile program (SPMD).

v2 schedule: the ScalarE exp stream (256 x [128,1024] activations) is the
binding engine together with PE; the program is organized as one flat loop of
256 attention steps (pair p, 512-token window w, key-chunk m) so that:
  - exp starts ~15us in: only pair-0's Q/K projection gates it (weights are
    host-split so those columns DMA first);
  - all remaining projection work (Q/K of pairs 1-3, all of V) trickles into
    per-step PE slack through a dedicated 1-bank PSUM ring;
  - AV matmuls trail the exp stream by 12 steps (pt ring keeps 16 tiles), so
    V-projection in the first 16 steps never stalls ScalarE;
  - S matmuls for the two heads of a pair use PE row-tiles (0,0)/(64,0)
    (K=64 each) so they can run concurrently on hardware.
Scores are computed transposed (S.T = K.T' @ Q.T per head); exp on ScalarE
(no max subtraction: |S|*0.125 <= ~6); V carries an appended ones column so
softmax denominators fall out of the AV matmul; a PE transpose + per-row
1/den scaling produces the output in natural layout.  All matmuls run in
bf16 with fp32 PSUM accumulation.
"""

from contextlib import ExitStack

import ml_dtypes
import numpy as np

import concourse.bass as bass
import concourse.mybir as mybir
import concourse.tile as tile
from concourse import bacc
from concourse.masks import make_identity

dt = mybir.dt
AF = mybir.ActivationFunctionType
BF16 = dt.bfloat16
F32 = dt.float32

B, N_TOK, C_IN = 4, 2048, 1024
NH = 8            # heads per core
NPAIR = NH // 2   # head pairs
D = 64
KC = C_IN // 128  # contraction k-tiles
MT = N_TOK // 128 # key-token chunks per window loop
NW = N_TOK // 512 # 512-token output windows per pair
VROW = 65         # V columns per head incl. ones column
NSTEP = NPAIR * NW * MT  # 256 flat attention steps
NWIN = NPAIR * NW        # 16 (pair, window) units
# AV trails exp: enough slack early on that trickled V-projection never
# stalls ScalarE, tapering late so the post-loop AV flush tail is short.
# av PSUM ring-1 requires TRAIL[k+1] >= TRAIL[k] - 1.
TRAIL_BY_WIN = [max(10, 23 - max(0, k - 2)) for k in range(NWIN)]


def build_nc(iters: int = 1):
    nc = bacc.Bacc(trn_type="TRN2")
    xT = nc.dram_tensor("xT", [C_IN, N_TOK], BF16, kind="ExternalInput").ap()
    wT1 = nc.dram_tensor("wT1", [C_IN, 256], BF16, kind="ExternalInput").ap()
    wT2 = nc.dram_tensor("wT2", [C_IN, 768], BF16, kind="ExternalInput").ap()
    wTv = nc.dram_tensor("wTv", [C_IN, 512], BF16, kind="ExternalInput").ap()
    qkb = nc.dram_tensor("qkb", [1024], F32, kind="ExternalInput").ap()
    vb = nc.dram_tensor("vb", [512], F32, kind="ExternalInput").ap()
    out = nc.dram_tensor("out", [N_TOK, NH * D], F32, kind="ExternalOutput").ap()

    with tile.TileContext(nc) as tc, ExitStack() as ctx:
        consts = ctx.enter_context(tc.tile_pool(name="consts", bufs=1))
        p_xt = ctx.enter_context(tc.tile_pool(name="p_xt", bufs=1))
        p_w1 = ctx.enter_context(tc.tile_pool(name="p_w1", bufs=1))
        p_w2 = ctx.enter_context(tc.tile_pool(name="p_w2", bufs=1))
        p_wv = ctx.enter_context(tc.tile_pool(name="p_wv", bufs=1))
        p_qk = ctx.enter_context(tc.tile_pool(name="p_qk", bufs=2 * NPAIR))
        p_vp = ctx.enter_context(tc.tile_pool(name="p_vp", bufs=MT))
        p_pt = ctx.enter_context(tc.tile_pool(name="p_pt", bufs=32))
        p_osb = ctx.enter_context(tc.tile_pool(name="p_osb", bufs=4))
        p_eps = ctx.enter_context(tc.tile_pool(name="p_eps", bufs=4))

        identity = consts.tile([128, 128], BF16, name="identity")
        make_identity(nc, identity)
        qkb_sb = consts.tile([128, 8], F32, name="qkb_sb")
        nc.sync.dma_start(out=qkb_sb, in_=qkb.rearrange("(t p) -> p t", p=128))
        vb_bc = consts.tile([128, 512], F32, name="vb_bc")
        nc.sync.dma_start(
            out=vb_bc,
            in_=bass.AP(tensor=vb.tensor, offset=vb.offset, ap=[[0, 128], vb.ap[0]]),
        )
        # preload the exp table set while input DMAs run
        warm = consts.tile([128, 1], F32, name="warm")
        nc.scalar.activation(out=warm, in_=qkb_sb[:, 0:1], func=AF.Exp)

        def body():
            # ---- input DMAs: one multi-dim-AP DMA per tensor (queue/SEQ
            # cost per dma_start is ~650ns; merging is a big head saving),
            # ordered by first use: w1+xtA gate the first S, wv gates the
            # trickled V units (steps 6+), xtB gates kU(0,2/3) (steps 2/4),
            # w2 gates pair-1 units (step 22+). ----
            w1t = p_w1.tile([128, KC * 256], BF16, name="w1", tag="w1")
            nc.sync.dma_start(
                out=w1t.rearrange("p (kc c) -> p kc c", kc=KC),
                in_=wT1.rearrange("(kc p) c -> p kc c", p=128),
            )
            xtt = p_xt.tile([128, KC * N_TOK], BF16, name="xt", tag="xt")
            xt3_sb = xtt.rearrange("p (kc c) -> p kc c", kc=KC)
            xt3_hbm = xT.rearrange("(kc p) c -> p kc c", p=128)
            nc.sync.dma_start(out=xt3_sb[:, :, 0:512], in_=xt3_hbm[:, :, 0:512])
            nc.sync.dma_start(out=xt3_sb[:, :, 512:1024], in_=xt3_hbm[:, :, 512:1024])
            wvt = p_wv.tile([128, KC * 512], BF16, name="wv", tag="wv")
            nc.sync.dma_start(
                out=wvt.rearrange("p (kc c) -> p kc c", kc=KC),
                in_=wTv.rearrange("(kc p) c -> p kc c", p=128),
            )
            nc.sync.dma_start(out=xt3_sb[:, :, 1024:2048], in_=xt3_hbm[:, :, 1024:2048])
            w2t = p_w2.tile([128, KC * 768], BF16, name="w2", tag="w2")
            nc.sync.dma_start(
                out=w2t.rearrange("p (kc c) -> p kc c", kc=KC),
                in_=wT2.rearrange("(kc p) c -> p kc c", p=128),
            )

            def xts(kc, sl):
                return xtt[:, kc * N_TOK + sl.start : kc * N_TOK + sl.stop]

            wv = [wvt[:, kc * 512 : (kc + 1) * 512] for kc in range(KC)]

            # persistent SBUF tiles
            qk = [
                p_qk.tile([128, N_TOK], BF16, name=f"qk{u}", tag="qk")
                for u in range(2 * NPAIR)
            ]
            vp = [
                p_vp.tile([128, NH * VROW], BF16, name=f"vp{tt}", tag="vp")
                for tt in range(MT)
            ]

            def w_slice(u, kc):
                # unit u: 0=q0,1=k0 from wT1; 2..7 = q1,k1,q2,k2,q3,k3 from wT2
                if u < 2:
                    return w1t[:, kc * 256 + u * 128 : kc * 256 + (u + 1) * 128]
                base = kc * 768 + (u - 2) * 128
                return w2t[:, base : base + 128]

            def qk_unit(pp_proj, u, tb):
                pj = pp_proj.tile([128, 512], F32, name=f"pj{u}_{tb}", tag="pj")
                for kc in range(KC):
                    nc.tensor.matmul(
                        pj,
                        lhsT=w_slice(u, kc),
                        rhs=xts(kc, slice(tb * 512, (tb + 1) * 512)),
                        start=(kc == 0),
                        stop=(kc == KC - 1),
                    )
                nc.vector.tensor_scalar_add(
                    out=qk[u][:, tb * 512 : (tb + 1) * 512],
                    in0=pj,
                    scalar1=qkb_sb[:, u : u + 1],
                )

            def v_unit(pp_proj, tt):
                pj = pp_proj.tile([128, 512], F32, name=f"pv{tt}", tag="pj")
                for kc in range(KC):
                    nc.tensor.matmul(
                        pj,
                        lhsT=xts(kc, slice(tt * 128, (tt + 1) * 128)),
                        rhs=wv[kc],
                        start=(kc == 0),
                        stop=(kc == KC - 1),
                    )
                t = vp[tt]
                t3 = t.rearrange("p (h d) -> p h d", h=NH)
                nc.gpsimd.memset(t3[:, :, 64:65], 1.0)
                nc.vector.tensor_add(
                    out=t3[:, :, 0:64],
                    in0=pj.rearrange("p (h d) -> p h d", h=NH),
                    in1=vb_bc.rearrange("p (h d) -> p h d", h=NH),
                )

            # trickle schedule: flat step -> list of (kind, args)
            emits = {}

            def emit_at(i, item):
                emits.setdefault(i, []).append(item)

            emit_at(2, ("qk", 1, 2))
            emit_at(4, ("qk", 1, 3))
            for t in range(MT):
                emit_at(6 + 2 * t, ("v", t))  # even steps 6..36; deadline t+TRAIL
            emit_at(9, ("qk", 0, 1))
            emit_at(25, ("qk", 0, 2))
            emit_at(41, ("qk", 0, 3))
            for p in range(1, NPAIR):
                for w in range(1, NW):
                    emit_at(64 * p + 16 * w - 8, ("qk", 2 * p, w))
                emit_at(64 * p - 41, ("qk", 2 * p, 0))
                for tb in range(NW):
                    emit_at(64 * p - 33 + 8 * tb, ("qk", 2 * p + 1, tb))

            with tc.tile_pool(name="pp_s", bufs=2, space="PSUM") as pp_s, \
                 tc.tile_pool(name="pp_av", bufs=1, space="PSUM") as pp_av, \
                 tc.tile_pool(name="pp_proj", bufs=1, space="PSUM") as pp_proj, \
                 tc.tile_pool(name="pp_tr", bufs=1, space="PSUM") as pp_tr:

                def epilogue(k, av_t):
                    p, w = divmod(k, NW)
                    for hh in range(2):
                        h = 2 * p + hh
                        osb = p_osb.tile([VROW, 512], BF16, name=f"osb{k}_{hh}", tag="osb")
                        nc.vector.tensor_copy(out=osb, in_=av_t[hh])
                        ob = p_eps.tile([128, 256], F32, name="ob", tag="ob")
                        for c in range(4):
                            tr = pp_tr.tile([128, VROW], BF16, name="tr", tag="tr")
                            nc.tensor.transpose(
                                tr,
                                in_=osb[:, c * 128 : (c + 1) * 128],
                                identity=identity[0:VROW, 0:VROW],
                            )
                            rc = p_eps.tile([128, 1], F32, name="rc", tag="rc")
                            nc.vector.reciprocal(out=rc, in_=tr[:, 64:65])
                            nc.vector.tensor_scalar_mul(
                                out=ob[:, c * 64 : (c + 1) * 64],
                                in0=tr[:, 0:64],
                                scalar1=rc,
                            )
                        # one DMA for the whole [512-token, 64-dim] block,
                        # issued from the Activation HWDGE queue (SP's is busy)
                        dst = out[w * 512 : (w + 1) * 512, h * 64 : (h + 1) * 64]
                        nc.scalar.dma_start(
                            out=dst.rearrange("(c p) d -> p c d", p=128),
                            in_=ob.rearrange("p (c d) -> p c d", c=4),
                        )

                # head units: pair-0 Q window 0 + K for tokens 0:1024
                qk_unit(pp_proj, 0, 0)
                qk_unit(pp_proj, 1, 0)
                qk_unit(pp_proj, 1, 1)

                pt_ring = [None] * NSTEP
                av_by_win = {}
                # av_emits[i] = AV steps to issue after S/exp of flat step i
                av_emits = {}
                for j in range(NSTEP):
                    k = j // MT
                    av_emits.setdefault(j + TRAIL_BY_WIN[k], []).append(j)

                def av_step(j):
                    p, rem = divmod(j, NW * MT)
                    w, m = divmod(rem, MT)
                    k = p * NW + w
                    if m == 0:
                        av_by_win[k] = [
                            pp_av.tile([VROW, 512], F32, name=f"av{k}_{hh}",
                                       tag=f"av{hh}")
                            for hh in range(2)
                        ]
                    av_t = av_by_win[k]
                    pt = pt_ring[j]
                    for hh in range(2):
                        h = 2 * p + hh
                        nc.tensor.matmul(
                            av_t[hh],
                            lhsT=vp[m][:, h * VROW : (h + 1) * VROW],
                            rhs=pt[:, hh * 512 : (hh + 1) * 512],
                            start=(m == 0),
                            stop=(m == MT - 1),
                        )
                    pt_ring[j] = None
                    if m == MT - 1:
                        epilogue(k, av_t)

                for i in range(NSTEP):
                    p, rem = divmod(i, NW * MT)
                    w, m = divmod(rem, MT)
                    # S + exp for step i
                    s = pp_s.tile([128, 1024], F32, name="s", tag="s")
                    for hh in range(2):
                        rows = slice(hh * 64, (hh + 1) * 64)
                        nc.tensor.matmul(
                            s[:, hh * 512 : (hh + 1) * 512],
                            lhsT=qk[2 * p + 1][rows, m * 128 : (m + 1) * 128],
                            rhs=qk[2 * p][rows, w * 512 : (w + 1) * 512],
                            start=True,
                            stop=True,
                            tile_position=(hh * 64, 0),
                        )
                    pt = p_pt.tile([128, 1024], BF16, name=f"pt{i % 16}", tag="pt")
                    nc.scalar.activation(out=pt, in_=s, func=AF.Exp, scale=0.125)
                    pt_ring[i] = pt
                    # trickled projection work
                    for item in emits.get(i, ()):
                        if item[0] == "v":
                            v_unit(pp_proj, item[1])
                        else:
                            qk_unit(pp_proj, item[1], item[2])
                    # trailing AV
                    for j in av_emits.get(i, ()):
                        av_step(j)
                for i in range(NSTEP, NSTEP + TRAIL_BY_WIN[-1]):
                    for j in av_emits.get(i, ()):
                        av_step(j)

        for _ in range(iters):
            body()

    nc.finalize()
    return nc


_NC_CACHE = {}


def _get_nc(iters: int = 1):
    if iters not in _NC_CACHE:
        _NC_CACHE[iters] = build_nc(iters)
    return _NC_CACHE[iters]


def make_in_maps(x, qkv_w, qkv_b):
    bf = ml_dtypes.bfloat16
    in_maps = []
    for core in range(8):
        b, g = core // 2, core % 2
        xTc = np.ascontiguousarray(x[b].T).astype(bf)
        wq = qkv_w[g * 512 : (g + 1) * 512]
        wk = qkv_w[1024 + g * 512 : 1024 + (g + 1) * 512]
        wv = qkv_w[2048 + g * 512 : 2048 + (g + 1) * 512]
        bq = qkv_b[g * 512 : (g + 1) * 512]
        bk = qkv_b[1024 + g * 512 : 1024 + (g + 1) * 512]
        bv = qkv_b[2048 + g * 512 : 2048 + (g + 1) * 512]
        w1 = np.concatenate([wq[0:128], wk[0:128]], axis=0)
        w2 = np.concatenate(
            [arr for f in range(1, 4) for arr in (wq[f * 128 : (f + 1) * 128],
                                                  wk[f * 128 : (f + 1) * 128])],
            axis=0,
        )
        qkbc = np.concatenate(
            [arr for f in range(4) for arr in (bq[f * 128 : (f + 1) * 128],
                                               bk[f * 128 : (f + 1) * 128])]
        )
        in_maps.append(
            {
                "xT": xTc,
                "wT1": np.ascontiguousarray(w1.T).astype(bf),
                "wT2": np.ascontiguousarray(w2.T).astype(bf),
                "wTv": np.ascontiguousarray(wv.T).astype(bf),
                "qkb": np.ascontiguousarray(qkbc).astype(np.float32),
                "vb": np.ascontiguousarray(bv).astype(np.float32),
            }
        )
    return in_maps


_RUNNER_CACHE = {}


def _get_runner(iters: int = 1, n_cores: int = 8):
    """Build the shard_map-wrapped bass_exec executable once and reuse it, so
    repeated kernel() calls don't re-ship the NEFF through the axon tunnel."""
    if iters in _RUNNER_CACHE:
        return _RUNNER_CACHE[iters]
    import jax
    from jax.sharding import Mesh, PartitionSpec
    from jax.experimental.shard_map import shard_map
    from concourse.bass2jax import (
        _bass_exec_p,
        install_neuronx_cc_hook,
        partition_id_tensor,
    )

    nc = _get_nc(iters)
    install_neuronx_cc_hook()
    partition_name = nc.partition_id_tensor.name if nc.partition_id_tensor else None
    in_names, out_names, out_avals, zero_outs = [], [], [], []
    for alloc in nc.m.functions[0].allocations:
        if not isinstance(alloc, mybir.MemoryLocationSet):
            continue
        name = alloc.memorylocations[0].name
        if alloc.kind == "ExternalInput":
            if name != partition_name:
                in_names.append(name)
        elif alloc.kind == "ExternalOutput":
            shape = tuple(alloc.tensor_shape)
            npdt = dt.np(alloc.dtype)
            out_names.append(name)
            out_avals.append(jax.core.ShapedArray(shape, npdt))
            zero_outs.append(np.zeros(shape, npdt))
    n_params = len(in_names)
    all_in_names = list(in_names) + list(out_names)
    if partition_name is not None:
        all_in_names.append(partition_name)

    def _body(*args):
        operands = list(args)
        if partition_name is not None:
            operands.append(partition_id_tensor())
        return tuple(
            _bass_exec_p.bind(
                *operands,
                out_avals=tuple(out_avals),
                in_names=tuple(all_in_names),
                out_names=tuple(out_names),
                lowering_input_output_aliases=(),
                sim_require_finite=True,
                sim_require_nnan=True,
                nc=nc,
            )
        )

    devices = jax.devices()[:n_cores]
    mesh = Mesh(np.asarray(devices), ("core",))
    in_specs = (PartitionSpec("core"),) * (n_params + len(out_names))
    out_specs = (PartitionSpec("core"),) * len(out_names)
    fn = jax.jit(
        shard_map(_body, mesh=mesh, in_specs=in_specs, out_specs=out_specs, check_rep=False)
    )
    zero_concat = [
        np.zeros((n_cores * z.shape[0], *z.shape[1:]), z.dtype) for z in zero_outs
    ]
    _RUNNER_CACHE[iters] = (fn, in_names, zero_concat, mesh)
    return _RUNNER_CACHE[iters]


def kernel(x, qkv_w, qkv_b):
    import jax

    x = np.asarray(x, dtype=np.float32)
    qkv_w = np.asarray(qkv_w, dtype=np.float32)
    qkv_b = np.asarray(qkv_b, dtype=np.float32)
    in_maps = make_in_maps(x, qkv_w, qkv_b)
    fn, in_names, zero_concat, _ = _get_runner(1)
    concat_in = [
        np.concatenate([in_maps[c][name] for c in range(8)], axis=0) for name in in_names
    ]
    outs = fn(*concat_in, *zero_concat)
    out_global = np.asarray(jax.block_until_ready(outs)[0])
    full = np.empty((B, N_TOK, C_IN), dtype=np.float32)
    for core in range(8):
        b, g = core // 2, core % 2
        full[b, :, g * 512 : (g + 1) * 512] = out_global[core * N_TOK : (core + 1) * N_TOK]
    return full
